# revision 37
# baseline (speedup 1.0000x reference)
"""MaskLinear kernel for 8x TRN2 NeuronCores.

Computes out[m,d] = sum_n weight[n] * masks[m,n] * x[n,d] + bias
 (= (masks * weight) @ x + bias), with x:[100000,256], masks:[64,100000].

Strategy: shard the contraction axis N across 8 cores. Each core gets a
12500-row slice (zero-padded to 12544 = 98*128 rows = "chunks" of 128),
computes a partial [2M,D] (two psum banks summed on-device), and the
host sums the 8 cores' [2M,D] halves and adds bias.

The host pre-folds weight into the transposed masks (wm = masks.T * w,
prescaled by 2^8 so fp16 stays in the normal range) so the device does
no elementwise work beyond the final bank add: each chunk is a pair of
f16 matmuls (lhsT=[128,64] wm, rhs=[128,256] x) run concurrently in PE
column groups 0-1/2-3. wm lives in its OWN SBUF tiles - if lhsT were a
slice of the same tile as rhs, LDWEIGHTS would contend with the running
matmul's rhs reads on the SBUF ports and the pair would serialize
(426ns/pair instead of 213).

DMA plan: ALL wm ships upfront as two ~0.8MB DMAs (one per HWDGE ring),
then the x stream follows in ramped groups alternating between the two
rings (byte-balanced; both rings end with small groups so neither limps
alone at the stream end). With wm pre-staged, the x-only tail delivers
a chunk pair every ~305ns while the PE consumes one every ~213ns, so
the PE has no backlog when the last bytes land. Chunks 0..49 accumulate
into psum bank A (parked in SBUF mid-stream - a DVE op can read at most
one PSUM operand); chunks 50..97 into bank B. The final chain after the
last x DMA is: pair -> add(psB + parked A) -> two parallel half DMAs.
"""

import numpy as np

import concourse.bacc as bacc
import concourse.mybir as mybir
from concourse import tile
from concourse.bass_utils import run_bass_kernel_spmd

N_CORES = 8
N = 100000
D = 256
M = 64
NS = N // N_CORES          # 12500 rows per shard
CHUNK = 128                # matmul contraction tile (partition dim)
C = -(-NS // CHUNK)        # 98 chunks
NP = C * CHUNK             # 12544 padded rows per shard
WSCALE = 256.0             # fp16 weight prescale (undone on host)

# x-stream groups (in chunks) and their HWDGE ring ("s"=sync, "c"=scalar).
# Ramped tail so the post-last-DMA matmul backlog is tiny; rings byte-
# balanced including the upfront wm DMAs (sync wmA=50, scalar wmB=48).
GROUPS = [14, 14, 14, 8, 14, 14, 6, 6, 4, 2, 2]
ENGS = ["s", "c", "c", "s", "s", "c", "s", "c", "s", "c", "s"]
# PRECTX (tried 3x, dead - do NOT re-enable): a raw pre-context wm DMA
# would start its decode ~0.7us before the entry barrier, but raw sync
# dma_start uses a DIFFERENT ring than the tile framework's sync DMAs, so
# "x tiles queue behind wm on the same ring" never holds: the wm transfer
# races the x stream and mid-stream pairs read unlanded weights (rel err
# 0.2-0.4 depending on SBUF placement; offset 184320 additionally landed
# in the top-down pool region -> NaN). A correct version needs an explicit
# cross-ring wmsem wait on the PE queue, with its own scheduler risks.
PRECTX = False
SPLIT = 3                  # groups[:SPLIT+1] (50 chunks) -> bank A
# HAM warmup (junk matmuls before the stream) was removed: with wm staged
# upfront it measured NEUTRAL-to-HARMFUL (epoch-controlled A/B, 6 pairs,
# mean -1.6us without it) - burning PE activity early appears to misalign
# the fixed-width HAM boost windows away from the tail pairs, which is
# where the clock state actually matters.
NWARM = 0

assert sum(GROUPS) == C
assert all(g % 2 == 0 for g in GROUPS)
A_CHUNKS = sum(GROUPS[:SPLIT + 1])
assert A_CHUNKS == 50

_STATE = {}


def _build_nc():
    nc = bacc.Bacc("TRN2", target_bir_lowering=False, debug=False,
                   num_devices=N_CORES)

    f16 = mybir.dt.float16
    f32 = mybir.dt.float32
    bf16 = mybir.dt.bfloat16

    # pk cols: [ wmA 50*64 | wmB 48*64 | x blocks per group, launch order ]
    WMA = A_CHUNKS * M
    WMB = (C - A_CHUNKS) * M
    XOFF = WMA + WMB
    pk = nc.dram_tensor("pk", [CHUNK, C * (M + D)], f16, kind="ExternalInput")
    out = nc.dram_tensor("out", [2 * M, D], f16, kind="ExternalOutput")

    if PRECTX:
        # Fixed mid-SBUF address (12544B at 100KB): clear of the tile pools
        # whether they allocate bottom-up (~16.5K..69K) or top-down
        # (~177K..229K). Offset 184320 hit the top-down pool region (NaN);
        # a bump allocation at base got overlapped too (rel err 0.4).
        wmflat = nc.alloc_sbuf_tensor_at(
            "wmflat", [CHUNK, C * M], f16, offset=102400)
        wmsem = nc.alloc_semaphore("wmflat_sem")
        nc.sync.dma_start(wmflat.ap(), pk[:, 0:XOFF]).then_inc(wmsem, 16)

    with tile.TileContext(nc) as tc:
        with (
            tc.tile_pool(name="cn", bufs=1) as cn,
            tc.tile_pool(name="gp", bufs=1) as gp,
            tc.tile_pool(name="pp", bufs=1, space="PSUM") as pp,
            tc.tile_pool(name="op", bufs=1) as op,
        ):
            if not PRECTX:
                # wm for the whole shard, staged before any x traffic.
                wmA = gp.tile([CHUNK, WMA], f16, tag="wmA")
                wmB = gp.tile([CHUNK, WMB], f16, tag="wmB")
                nc.sync.dma_start(wmA[:], pk[:, 0:WMA])
                nc.scalar.dma_start(wmB[:], pk[:, WMA:XOFF])

            if NWARM:
                # HAM warmup: junk bf16 matmuls keep the PE array busy while
                # the wm DMAs are in flight so the clock gate stays open
                # until real pairs arrive.
                jz = cn.tile([CHUNK, 512], bf16)
                wz = cn.tile([CHUNK, 1], bf16)
                nc.vector.memset(jz[:], 0.0)
                nc.vector.memset(wz[:], 0.0)
                pwarm = pp.tile([1, 512], f32, tag="pwarm")
                for i in range(NWARM):
                    nc.tensor.matmul(pwarm[:], wz[:], jz[:],
                                     start=(i == 0), stop=(i == NWARM - 1))

            psA = pp.tile([2 * M, D], f32, tag="psA")
            psB = pp.tile([2 * M, D], f32, tag="psB")
            n_pairs_A = A_CHUNKS // 2
            n_pairs_B = (C - A_CHUNKS) // 2

            cbase = 0
            for g, B in enumerate(GROUPS):
                xt = gp.tile([CHUNK, B * D], f16, tag=f"px{g}")
                if PRECTX:
                    eng = nc.sync   # same ring as wmflat: order = dependency
                else:
                    eng = nc.sync if ENGS[g] == "s" else nc.scalar
                off = XOFF + cbase * D
                eng.dma_start(xt[:], pk[:, off:off + B * D])

                for b in range(0, B, 2):
                    c = cbase + b
                    if c < A_CHUNKS:
                        ps, cp, np_ = psA, c // 2, n_pairs_A
                    else:
                        ps, cp, np_ = psB, (c - A_CHUNKS) // 2, n_pairs_B
                    if PRECTX:
                        wm, wc = wmflat, c
                    elif c < A_CHUNKS:
                        wm, wc = wmA, c
                    else:
                        wm, wc = wmB, c - A_CHUNKS
                    # Chunk pair: col groups 0-1 and 2-3 run concurrently,
                    # accumulating into disjoint psum partition halves.
                    nc.tensor.matmul(
                        ps[0:M, :],
                        wm[:, wc * M:(wc + 1) * M],
                        xt[:, b * D:(b + 1) * D],
                        start=(cp == 0),
                        stop=(cp == np_ - 1),
                        tile_position=(0, 0),
                    )
                    nc.tensor.matmul(
                        ps[M:2 * M, :],
                        wm[:, (wc + 1) * M:(wc + 2) * M],
                        xt[:, (b + 1) * D:(b + 2) * D],
                        start=(cp == 0),
                        stop=(cp == np_ - 1),
                        tile_position=(0, M),
                    )
                cbase += B
                if g == SPLIT:
                    # Bank A done: park it in SBUF (DVE is idle mid-stream).
                    osbA = op.tile([2 * M, D], f32, tag="osbA")
                    nc.vector.tensor_copy(osbA[:], psA[:])
            # Final drain: bank A never round-trips to DRAM - the last DVE
            # op adds bank B (PSUM) to the parked bank A (same cost as a
            # copy: free size drives the cycles; Pool cannot access PSUM so
            # the add cannot be split across engines), then two parallel
            # half DMAs on sync+scalar move the single [128,256] result.
            # The add writes f16 to halve the output DMA: the per-core
            # partial at f16 costs ~2e-4 extra rel err, far inside budget.
            osbB = op.tile([2 * M, D], f16, tag="osbB")
            nc.vector.tensor_add(osbB[:], psB[:], osbA[:])
            nc.sync.dma_start(out[0:M, :], osbB[0:M, :])
            nc.scalar.dma_start(out[M:2 * M, :], osbB[M:2 * M, :])
    nc.compile()
    return nc


def _get_nc():
    if "nc" not in _STATE:
        _STATE["nc"] = _build_nc()
    return _STATE["nc"]


def _shard_inputs(x, masks, weight):
    dt = np.dtype(np.float16)
    x = np.asarray(x, dtype=np.float32)
    masks = np.asarray(masks, dtype=np.float32)
    weight = np.asarray(weight, dtype=np.float32)
    # Fold the weight into the transposed masks on the host; fp16 prescale
    # by 2**8 keeps the tiny products (~1/sqrt(N)) in the normal range
    # (exact, undone after the gather).
    wmasks = masks.T * (weight * WSCALE)[:, None]   # [N, M] f32

    in_maps = []
    for s in range(N_CORES):
        lo = s * NS
        hi = lo + NS
        xs = np.zeros((NP, D), dt)
        xs[:NS] = x[lo:hi].astype(dt, copy=False)
        ms = np.zeros((NP, M), dt)
        ms[:NS] = wmasks[lo:hi].astype(dt, copy=False)
        # Pack per group: row (cbase*128 + p*B + b) lands on partition p as
        # sub-chunk b (same permutation for wm and x, so the contraction is
        # unaffected). All wm blocks first (staged upfront), then x blocks.
        wm_blocks = []
        x_blocks = []
        cbase = 0
        for B in GROUPS:
            r0, r1 = cbase * CHUNK, (cbase + B) * CHUNK
            wm_blocks.append(ms[r0:r1].reshape(CHUNK, B * M))
            x_blocks.append(xs[r0:r1].reshape(CHUNK, B * D))
            cbase += B
        pkv = np.concatenate(wm_blocks + x_blocks, axis=1)
        assert pkv.shape == (CHUNK, C * (M + D))
        in_maps.append({"pk": pkv})
    return in_maps


def _run(x, masks, weight, bias, **run_kwargs):
    in_maps = _shard_inputs(x, masks, weight)
    try:
        res = run_bass_kernel_spmd(
            _get_nc(), in_maps, core_ids=list(range(N_CORES)), **run_kwargs
        )
    except Exception:
        # The runtime occasionally reports a transient unrecoverable-device
        # error that clears on the next execution; retry once.
        res = run_bass_kernel_spmd(
            _get_nc(), in_maps, core_ids=list(range(N_CORES)), **run_kwargs
        )
    parts = np.stack([r["out"] for r in res.results]).astype(np.float32)
    full = parts.sum(axis=0)                           # [2M, 256]
    full = full[:M] + full[M:2 * M]
    full = full * np.float32(1.0 / WSCALE)
    out = full + np.asarray(bias, dtype=np.float32)
    return out.astype(np.float32), res


def kernel(x, masks, weight, bias):
    out, _ = _run(x, masks, weight, bias)
    return out


# revision 40
# speedup vs baseline: 1.0742x; 1.0742x over previous
"""MaskLinear kernel for 8x TRN2 NeuronCores.

Computes out[m,d] = sum_n weight[n] * masks[m,n] * x[n,d] + bias
 (= (masks * weight) @ x + bias), with x:[100000,256], masks:[64,100000].

Strategy: shard the contraction axis N across 8 cores. Each core gets a
12500-row slice (zero-padded to 12544 = 98*128 rows = "chunks" of 128),
computes a partial [2M,D] (two psum banks summed on-device), and the
host sums the 8 cores' [2M,D] halves and adds bias.

The host pre-folds weight into the transposed masks (wm = masks.T * w,
prescaled by 2^8 so fp16 stays in the normal range) so the device does
no elementwise work beyond the final bank add: each chunk is a pair of
f16 matmuls (lhsT=[128,64] wm, rhs=[128,256] x) run concurrently in PE
column groups 0-1/2-3. wm lives in its OWN SBUF tiles - if lhsT were a
slice of the same tile as rhs, LDWEIGHTS would contend with the running
matmul's rhs reads on the SBUF ports and the pair would serialize
(426ns/pair instead of 213).

DMA plan: ALL wm ships upfront as two ~0.8MB DMAs (one per HWDGE ring),
then the x stream follows in ramped groups alternating between the two
rings (byte-balanced; both rings end with small groups so neither limps
alone at the stream end). With wm pre-staged, the x-only tail delivers
a chunk pair every ~305ns while the PE consumes one every ~213ns, so
the PE has no backlog when the last bytes land. Chunks 0..49 accumulate
into psum bank A (parked in SBUF mid-stream - a DVE op can read at most
one PSUM operand); chunks 50..97 into bank B. The final chain after the
last x DMA is: pair -> add(psB + parked A) -> two parallel half DMAs.
"""

import numpy as np

import concourse.bacc as bacc
import concourse.mybir as mybir
from concourse import tile
from concourse.bass_utils import run_bass_kernel_spmd

N_CORES = 8
N = 100000
D = 256
M = 64
NS = N // N_CORES          # 12500 rows per shard
CHUNK = 128                # matmul contraction tile (partition dim)
C = -(-NS // CHUNK)        # 98 chunks
NP = C * CHUNK             # 12544 padded rows per shard
WSCALE = 256.0             # fp16 weight prescale (undone on host)

# x-stream groups (in chunks) and their HWDGE ring ("s"=sync, "c"=scalar).
# Ramped tail so the post-last-DMA matmul backlog is tiny; rings byte-
# balanced including the upfront wm DMAs (sync wmA=50, scalar wmB=48).
GROUPS = [14, 14, 14, 8, 14, 14, 6, 6, 4, 2, 2]
ENGS = ["s", "c", "c", "s", "s", "c", "s", "c", "s", "c", "s"]
# PRECTX (tried 4x, conclusively dead - do NOT re-enable): issuing the wm
# DMA before the TileContext would start the stream ~0.7us earlier, but
# (a) raw dma_start rides a DIFFERENT DGE ring than the tile framework's
# DMAs, so ring-order arguments across the context boundary do not hold
# and mid-stream pairs race the wm transfer (rel err 0.2-0.4, placement-
# independent); (b) an explicit in-context wait_ge on the DMA's completion
# semaphore deadlocks the tile scheduler's simulation (the pre-context
# increment is invisible to it). The framework cannot express a
# dependency on an out-of-context event.
PRECTX = False
SPLIT = 3                  # groups[:SPLIT+1] (50 chunks) -> bank A
# HAM warmup (junk matmuls before the stream) was removed: with wm staged
# upfront it measured NEUTRAL-to-HARMFUL (epoch-controlled A/B, 6 pairs,
# mean -1.6us without it) - burning PE activity early appears to misalign
# the fixed-width HAM boost windows away from the tail pairs, which is
# where the clock state actually matters.
NWARM = 0

assert sum(GROUPS) == C
assert all(g % 2 == 0 for g in GROUPS)
A_CHUNKS = sum(GROUPS[:SPLIT + 1])
assert A_CHUNKS == 50

_STATE = {}


def _build_nc():
    nc = bacc.Bacc("TRN2", target_bir_lowering=False, debug=False,
                   num_devices=N_CORES)

    f16 = mybir.dt.float16
    f32 = mybir.dt.float32
    bf16 = mybir.dt.bfloat16

    # pk cols: [ wmA 50*64 | wmB 48*64 | x blocks per group, launch order ]
    WMA = A_CHUNKS * M
    WMB = (C - A_CHUNKS) * M
    XOFF = WMA + WMB
    pk = nc.dram_tensor("pk", [CHUNK, C * (M + D)], f16, kind="ExternalInput")
    out = nc.dram_tensor("out", [2 * M, D], f16, kind="ExternalOutput")

    if PRECTX:
        # Fixed mid-SBUF address (12544B at 100KB): clear of the tile pools
        # whether they allocate bottom-up (~16.5K..69K) or top-down
        # (~177K..229K). Offset 184320 hit the top-down pool region (NaN);
        # a bump allocation at base got overlapped too (rel err 0.4).
        wmflat = nc.alloc_sbuf_tensor_at(
            "wmflat", [CHUNK, C * M], f16, offset=102400)
        wmsem = nc.alloc_semaphore("wmflat_sem")
        nc.sync.dma_start(wmflat.ap(), pk[:, 0:XOFF]).then_inc(wmsem, 16)

    with tile.TileContext(nc) as tc:
        with (
            tc.tile_pool(name="cn", bufs=1) as cn,
            tc.tile_pool(name="gp", bufs=1) as gp,
            tc.tile_pool(name="pp", bufs=1, space="PSUM") as pp,
            tc.tile_pool(name="op", bufs=1) as op,
        ):
            if not PRECTX:
                # wm for the whole shard, staged before any x traffic.
                wmA = gp.tile([CHUNK, WMA], f16, tag="wmA")
                wmB = gp.tile([CHUNK, WMB], f16, tag="wmB")
                nc.sync.dma_start(wmA[:], pk[:, 0:WMA])
                nc.scalar.dma_start(wmB[:], pk[:, WMA:XOFF])

            if NWARM:
                # HAM warmup: junk bf16 matmuls keep the PE array busy while
                # the wm DMAs are in flight so the clock gate stays open
                # until real pairs arrive.
                jz = cn.tile([CHUNK, 512], bf16)
                wz = cn.tile([CHUNK, 1], bf16)
                nc.vector.memset(jz[:], 0.0)
                nc.vector.memset(wz[:], 0.0)
                pwarm = pp.tile([1, 512], f32, tag="pwarm")
                for i in range(NWARM):
                    nc.tensor.matmul(pwarm[:], wz[:], jz[:],
                                     start=(i == 0), stop=(i == NWARM - 1))

            psA = pp.tile([2 * M, D], f32, tag="psA")
            psB = pp.tile([2 * M, D], f32, tag="psB")
            n_pairs_A = A_CHUNKS // 2
            n_pairs_B = (C - A_CHUNKS) // 2

            if PRECTX:
                # Gate the PE on the pre-context wm DMA (adds 16 at
                # completion). This is the ONLY ordering between the raw
                # ring and the PE - without it mid-stream pairs race the
                # wm transfer and read garbage.
                nc.tensor.wait_ge(wmsem, 16)

            cbase = 0
            for g, B in enumerate(GROUPS):
                xt = gp.tile([CHUNK, B * D], f16, tag=f"px{g}")
                eng = nc.sync if ENGS[g] == "s" else nc.scalar
                off = XOFF + cbase * D
                eng.dma_start(xt[:], pk[:, off:off + B * D])

                for b in range(0, B, 2):
                    c = cbase + b
                    if c < A_CHUNKS:
                        ps, cp, np_ = psA, c // 2, n_pairs_A
                    else:
                        ps, cp, np_ = psB, (c - A_CHUNKS) // 2, n_pairs_B
                    if PRECTX:
                        wm, wc = wmflat, c
                    elif c < A_CHUNKS:
                        wm, wc = wmA, c
                    else:
                        wm, wc = wmB, c - A_CHUNKS
                    # Chunk pair: col groups 0-1 and 2-3 run concurrently,
                    # accumulating into disjoint psum partition halves.
                    nc.tensor.matmul(
                        ps[0:M, :],
                        wm[:, wc * M:(wc + 1) * M],
                        xt[:, b * D:(b + 1) * D],
                        start=(cp == 0),
                        stop=(cp == np_ - 1),
                        tile_position=(0, 0),
                    )
                    nc.tensor.matmul(
                        ps[M:2 * M, :],
                        wm[:, (wc + 1) * M:(wc + 2) * M],
                        xt[:, (b + 1) * D:(b + 2) * D],
                        start=(cp == 0),
                        stop=(cp == np_ - 1),
                        tile_position=(0, M),
                    )
                cbase += B
                if g == SPLIT:
                    # Bank A done: park it in SBUF (DVE is idle mid-stream).
                    osbA = op.tile([2 * M, D], f32, tag="osbA")
                    nc.vector.tensor_copy(osbA[:], psA[:])
            # Final drain: bank A never round-trips to DRAM - the last DVE
            # op adds bank B (PSUM) to the parked bank A (same cost as a
            # copy: free size drives the cycles; Pool cannot access PSUM so
            # the add cannot be split across engines), then two parallel
            # half DMAs on sync+scalar move the single [128,256] result.
            # The add writes f16 to halve the output DMA: the per-core
            # partial at f16 costs ~2e-4 extra rel err, far inside budget.
            osbB = op.tile([2 * M, D], f16, tag="osbB")
            nc.vector.tensor_add(osbB[:], psB[:], osbA[:])
            nc.sync.dma_start(out[0:M, :], osbB[0:M, :])
            nc.scalar.dma_start(out[M:2 * M, :], osbB[M:2 * M, :])
    nc.compile()
    return nc


def _get_nc():
    if "nc" not in _STATE:
        _STATE["nc"] = _build_nc()
    return _STATE["nc"]


def _shard_inputs(x, masks, weight):
    dt = np.dtype(np.float16)
    x = np.asarray(x, dtype=np.float32)
    masks = np.asarray(masks, dtype=np.float32)
    weight = np.asarray(weight, dtype=np.float32)
    # Fold the weight into the transposed masks on the host; fp16 prescale
    # by 2**8 keeps the tiny products (~1/sqrt(N)) in the normal range
    # (exact, undone after the gather).
    wmasks = masks.T * (weight * WSCALE)[:, None]   # [N, M] f32

    in_maps = []
    for s in range(N_CORES):
        lo = s * NS
        hi = lo + NS
        xs = np.zeros((NP, D), dt)
        xs[:NS] = x[lo:hi].astype(dt, copy=False)
        ms = np.zeros((NP, M), dt)
        ms[:NS] = wmasks[lo:hi].astype(dt, copy=False)
        # Pack per group: row (cbase*128 + p*B + b) lands on partition p as
        # sub-chunk b (same permutation for wm and x, so the contraction is
        # unaffected). All wm blocks first (staged upfront), then x blocks.
        wm_blocks = []
        x_blocks = []
        cbase = 0
        for B in GROUPS:
            r0, r1 = cbase * CHUNK, (cbase + B) * CHUNK
            wm_blocks.append(ms[r0:r1].reshape(CHUNK, B * M))
            x_blocks.append(xs[r0:r1].reshape(CHUNK, B * D))
            cbase += B
        pkv = np.concatenate(wm_blocks + x_blocks, axis=1)
        assert pkv.shape == (CHUNK, C * (M + D))
        in_maps.append({"pk": pkv})
    return in_maps


def _run(x, masks, weight, bias, **run_kwargs):
    in_maps = _shard_inputs(x, masks, weight)
    try:
        res = run_bass_kernel_spmd(
            _get_nc(), in_maps, core_ids=list(range(N_CORES)), **run_kwargs
        )
    except Exception:
        # The runtime occasionally reports a transient unrecoverable-device
        # error that clears on the next execution; retry once.
        res = run_bass_kernel_spmd(
            _get_nc(), in_maps, core_ids=list(range(N_CORES)), **run_kwargs
        )
    parts = np.stack([r["out"] for r in res.results]).astype(np.float32)
    full = parts.sum(axis=0)                           # [2M, 256]
    full = full[:M] + full[M:2 * M]
    full = full * np.float32(1.0 / WSCALE)
    out = full + np.asarray(bias, dtype=np.float32)
    return out.astype(np.float32), res


def kernel(x, masks, weight, bias):
    out, _ = _run(x, masks, weight, bias)
    return out
